# revision 16
# baseline (speedup 1.0000x reference)
"""Trainium2 Bass kernel for nn_Block_70952859730367 (dense transformer block).

Strategy (8 NeuronCores, SPMD, one launch):
  Phase A  (per core): QKV projections for this core's 2 heads (h=2c, 2c+1)
           over ALL B*T tokens, in [d, t] layout (fp32r matmuls, K=C=1024).
  Phase A' : PE-transpose V into [s, d] layout + ones column (for softmax denom).
  Phase B  : causal attention per (b, 512-token t-block): scoresT = K^T-chunks
           vs Q (row-packed 2 heads, concurrent), exp on ACT (no max-sub — scores
           are small), causal mask via DVE multiply with host-fed mask tiles
           (diagonal chunks processed first), attnV accumulation with an appended
           ones column giving the softmax denominator for free.
           Normalize via DVE reciprocal + gpsimd partition_broadcast.
  A2A      : AllToAll redistributes attention outputs: core c ends up with all
           1024 head-dims for ITS 512 tokens.
  Phase D  : proj + residual + SiLU MLP + residual, token-parallel (512 tokens
           per core), streaming Wproj/W1/W2 slabs from HBM.

All matmuls run as float32r (TF32-rate: 1 cyc/row at N>=512) with fp32 PSUM
accumulation. Residual stream kept in full fp32.
"""
import os
import numpy as np

import concourse.bass as bass
import concourse.tile as tile
from concourse import bacc, mybir
from concourse import bass_utils
from concourse.masks import make_identity

B, T, C = 2, 2048, 1024
H, HS, FF = 16, 64, 4096
NT = B * T                      # 4096 tokens, b-major
NCORES = 8
TOK = NT // NCORES              # 512 tokens per core
SCALE = HS ** -0.5              # 0.125

F32 = mybir.dt.float32
F32R = mybir.dt.float32r
AF = mybir.ActivationFunctionType
ALU = mybir.AluOpType

_PROGRAM = None
LAST_EXEC_NS = None


def _emit(nc, tc, io, use_collective=True, stop_after=None):
    xT, xTown, wqkv, wproj, w1t, w2t, b1, out_d = (
        io["xT"], io["xTown"], io["wqkv"], io["wproj"], io["w1t"], io["w2t"],
        io["b1"], io["out"])
    masks = io["masks"]
    from contextlib import ExitStack

    outer = ExitStack()
    const = outer.enter_context(tc.tile_pool(name="const", bufs=1))
    wqkv_sb = const.tile([128, 8, 384], F32R, tag="wqkv")

    def load_wqkv_d(d):
        nc.sync.dma_start(
            out=wqkv_sb[:, :, 128 * d:128 * (d + 1)],
            in_=wqkv.ap().rearrange("(k p) d -> p k d", p=128)
                [:, :, 128 * d:128 * (d + 1)].bitcast(F32R))

    load_wqkv_d(0)   # q weights only; k/v weight loads go after tb0's x chunks
    identity = const.tile([128, 128], F32, tag="ident")
    make_identity(nc, identity[:])
    b1_sb = const.tile([128, 32, 1], F32, tag="b1")
    xTown_sb = const.tile([128, 8, 512], F32, tag="xTown")
    masks_sb = const.tile([128, 4, 512], F32, tag="masks")

    # W streaming pool — open early so prefetch DMAs can run during attention.
    wpool = outer.enter_context(tc.tile_pool(name="wstream", bufs=6))

    # DRAM bounce for the collective
    dram = outer.enter_context(tc.tile_pool(name="dram", bufs=1, space="DRAM"))
    a2a_in = dram.tile([8, 128, 512], F32, tag="a2ai")
    a2a_out = dram.tile([8, 128, 512], F32, tag="a2ao")

    attn_scope = ExitStack()
    qkvpool = attn_scope.enter_context(tc.tile_pool(name="qkv", bufs=1))
    q_sb = [qkvpool.tile([128, 2048], F32R, tag=f"q{b}", name=f"q{b}") for b in range(2)]
    k_sb = [qkvpool.tile([128, 2048], F32R, tag=f"k{b}", name=f"k{b}") for b in range(2)]
    v_sb = [qkvpool.tile([128, 2048], F32, tag=f"v{b}", name=f"v{b}") for b in range(2)]
    vsd = [qkvpool.tile([128, 16, 130], F32R, tag=f"vsd{b}", name=f"vsd{b}") for b in range(2)]

    # ---------------- Phase A: QKV + V-transpose ----------------
    with ExitStack() as pa:
        xtp = pa.enter_context(tc.tile_pool(name="xt", bufs=20))
        qkvp = pa.enter_context(tc.tile_pool(name="qkvp", bufs=6, space="PSUM"))
        tp = pa.enter_context(tc.tile_pool(name="tp", bufs=2, space="PSUM"))

        def emit_vtrans(b):
            for sc in range(16):
                pt = tp.tile([128, 128], F32, tag="tp", name=f"pt{b}_{sc}")
                nc.tensor.transpose(pt[:], v_sb[b][:, 128 * sc:128 * (sc + 1)],
                                    identity[:])
                dstv = vsd[b][:, sc, :].rearrange("p (h q) -> p h q", h=2)[:, :, 0:64]
                srcv = pt[:].rearrange("p (h q) -> p h q", h=2)
                nc.vector.tensor_copy(dstv, srcv)
            vv = vsd[b][:].rearrange("p k (h q) -> p k h q", h=2)
            nc.gpsimd.affine_select(
                out=vv, in_=vv, compare_op=ALU.not_equal, fill=1.0,
                base=-64, channel_multiplier=0,
                pattern=[[0, 16], [0, 2], [1, 65]])

        for tb in range(8):              # b-major 512-token blocks
            b, j = tb // 4, tb % 4
            xts = []
            for k in range(8):
                xt_t = xtp.tile([128, 512], F32R, tag="xt", name=f"xt{tb}_{k}")
                nc.sync.dma_start(
                    out=xt_t,
                    in_=xT.ap()[128 * k:128 * (k + 1),
                                512 * tb:512 * (tb + 1)].bitcast(F32R))
                xts.append(xt_t[:])
            if tb == 0:
                load_wqkv_d(1)
                load_wqkv_d(2)
            for d in range(3):           # q, k, v
                ps = qkvp.tile([128, 512], F32, tag="qkvp")
                for k in range(8):
                    nc.tensor.matmul(ps[:],
                                     lhsT=wqkv_sb[:, k, 128 * d:128 * (d + 1)],
                                     rhs=xts[k],
                                     start=(k == 0), stop=(k == 7))
                dst = (q_sb, k_sb, v_sb)[d][b]
                nc.vector.tensor_copy(dst[:, 512 * j:512 * (j + 1)], ps[:])
            if j == 3:
                emit_vtrans(b)

    if stop_after == "a":
        attn_scope.close()
        outer.close()
        return
    # ---------------- Phase B: attention ----------------
    with ExitStack() as pb:
        scp = pb.enter_context(tc.tile_pool(name="scp", bufs=2, space="PSUM"))
        avp = pb.enter_context(tc.tile_pool(name="avp", bufs=4, space="PSUM"))
        ep = pb.enter_context(tc.tile_pool(name="ep", bufs=8))
        afp = pb.enter_context(tc.tile_pool(name="afp", bufs=4))
        rp = pb.enter_context(tc.tile_pool(name="rp", bufs=4))

        nc.sync.dma_start(out=masks_sb, in_=masks.ap())

        for b in range(2):
            for j in range(4):
                t0 = 512 * j
                kmax = 4 * (j + 1)
                av = [avp.tile([65, 512], F32, tag="av", name=f"av{b}_{j}_{_h}") for _h in range(2)]

                def emit_attnv(pend):
                    # emit strictly descending in k so the start=True matmul
                    # (k = kmax-1) is the first in the PSUM accumulation group
                    # and stop=True (k = 0) is the last.
                    e, h, k0, k1 = pend
                    for ki, kk in ((1, k1), (0, k0)):
                        nc.tensor.matmul(
                            av[h][:],
                            lhsT=vsd[b][:, kk, 65 * h:65 * (h + 1)],
                            rhs=e[:, ki, :],
                            start=(kk == kmax - 1), stop=(kk == 0),
                            skip_group_check=True)

                pending = []
                npairs = kmax // 2
                for pr in range(npairs - 1, -1, -1):   # diag pairs first
                    k0, k1 = 2 * pr, 2 * pr + 1
                    for h in range(2):
                        sp = scp.tile([128, 2, 512], F32, tag="sc")
                        for ki, k in enumerate((k0, k1)):
                            nc.tensor.matmul(
                                sp[:, ki, :],
                                lhsT=k_sb[b][64 * h:64 * (h + 1),
                                             128 * k:128 * (k + 1)],
                                rhs=q_sb[b][64 * h:64 * (h + 1), t0:t0 + 512],
                                start=True, stop=True, skip_group_check=True)
                        e = ep.tile([128, 2, 512], F32R, tag="e")
                        nc.scalar.activation(e[:], sp[:], AF.Exp, scale=SCALE)
                        for ki, k in enumerate((k0, k1)):
                            if 128 * (k + 1) > t0:   # diagonal chunk: mask
                                nc.vector.tensor_mul(e[:, ki, :], e[:, ki, :],
                                                     masks_sb[:, k - 4 * j, :])
                        pending.append((e, h, k0, k1))
                    while len(pending) > 2:
                        emit_attnv(pending.pop(0))
                for p in pending:
                    emit_attnv(p)

                blk = 4 * b + j
                for h in range(2):
                    r = rp.tile([1, 512], F32, tag="r")
                    nc.vector.reciprocal(r[:], av[h][64:65, :])
                    rb = rp.tile([64, 512], F32, tag="rb")
                    nc.gpsimd.partition_broadcast(rb[:], r[:])
                    af = afp.tile([64, 512], F32, tag="af")
                    nc.vector.tensor_mul(af[:], av[h][0:64, :], rb[:])
                    nc.sync.dma_start(out=a2a_in[blk, 64 * h:64 * (h + 1), :],
                                      in_=af[:])

    if stop_after == "b":
        attn_scope.close()
        outer.close()
        return
    attn_scope.close()

    # Prefetch first weight slabs BEFORE the collective so the SP DMA queue
    # isn't head-of-line blocked behind collective-dependent loads.
    wp_pre = []
    for cc in range(6):
        wp = wpool.tile([128, 8, 128], F32R, tag="w", name=f"wpre{cc}")
        nc.sync.dma_start(
            out=wp,
            in_=wproj.ap().rearrange("(k p) m -> p k m", p=128)
                [:, :, 128 * cc:128 * (cc + 1)].bitcast(F32R))
        wp_pre.append(wp)

    # ---------------- A2A ----------------
    if use_collective:
        nc.gpsimd.collective_compute(
            "AllToAll", ALU.bypass,
            replica_groups=[list(range(NCORES))],
            ins=[a2a_in.opt()], outs=[a2a_out.opt()])
    else:  # timing-estimation build: stand-in DMA with similar byte volume
        nc.sync.dma_start(out=a2a_out[:], in_=a2a_in[:])

    if stop_after == "c":
        outer.close()
        return
    # ---------------- Phase D: proj + residual + MLP ----------------
    with ExitStack() as pd:
        atnp = pd.enter_context(tc.tile_pool(name="atn", bufs=8))
        x2fp = pd.enter_context(tc.tile_pool(name="x2f", bufs=8))
        x2rp = pd.enter_context(tc.tile_pool(name="x2r", bufs=8))
        hp = pd.enter_context(tc.tile_pool(name="hp", bufs=32))
        outp = pd.enter_context(tc.tile_pool(name="outp", bufs=4))
        mmp = pd.enter_context(tc.tile_pool(name="mmp", bufs=3, space="PSUM"))

        nc.sync.dma_start(out=b1_sb,
                          in_=b1.ap().rearrange("(k p) o -> p k o", p=128))
        nc.sync.dma_start(out=xTown_sb,
                          in_=xTown.ap().rearrange("(k p) n -> p k n", p=128))
        atn = []
        for k in range(8):
            t = atnp.tile([128, 512], F32R, tag="atn", name=f"atn{k}")
            nc.gpsimd.dma_start(out=t, in_=a2a_out[k].bitcast(F32R))
            atn.append(t)

        # proj + residual
        x2f, x2r = [], []
        for cc in range(8):
            if cc < 6:
                wp = wp_pre[cc]
            else:
                wp = wpool.tile([128, 8, 128], F32R, tag="w", name=f"wp{cc}")
                nc.sync.dma_start(
                    out=wp,
                    in_=wproj.ap().rearrange("(k p) m -> p k m", p=128)
                        [:, :, 128 * cc:128 * (cc + 1)].bitcast(F32R))
            ps = mmp.tile([128, 512], F32, tag="mm")
            for k in range(8):
                nc.tensor.matmul(ps[:], lhsT=wp[:, k, :], rhs=atn[k][:],
                                 start=(k == 0), stop=(k == 7))
            xf = x2fp.tile([128, 512], F32, tag="x2f")
            nc.vector.tensor_add(xf[:], xTown_sb[:, cc, :], ps[:])
            xr = x2rp.tile([128, 512], F32R, tag="x2r")
            nc.vector.tensor_copy(xr[:], xf[:])
            x2f.append(xf)
            x2r.append(xr)

        # mm1 + silu
        hts = []
        for fc in range(32):
            wp = wpool.tile([128, 8, 128], F32R, tag="w")
            nc.sync.dma_start(
                out=wp,
                in_=w1t.ap().rearrange("(k p) m -> p k m", p=128)
                    [:, :, 128 * fc:128 * (fc + 1)].bitcast(F32R))
            ps = mmp.tile([128, 512], F32, tag="mm")
            for cc in range(8):
                nc.tensor.matmul(ps[:], lhsT=wp[:, cc, :], rhs=x2r[cc][:],
                                 start=(cc == 0), stop=(cc == 7))
            ht = hp.tile([128, 512], F32R, tag="h")
            nc.scalar.activation(ht[:], ps[:], AF.Silu, bias=b1_sb[:, fc, :])
            hts.append(ht)

        # mm2 + residual + output
        for cc in range(8):
            ps = mmp.tile([128, 512], F32, tag="mm")
            for quarter in range(4):
                w2p = wpool.tile([128, 8, 128], F32R, tag="w", name=f"w2q{cc}_{quarter}")
                nc.sync.dma_start(
                    out=w2p,
                    in_=w2t.ap().rearrange("(k p) m -> p k m", p=128)
                        [:, 8 * quarter:8 * (quarter + 1),
                         128 * cc:128 * (cc + 1)].bitcast(F32R))
                for f in range(8):
                    fc = 8 * quarter + f
                    nc.tensor.matmul(ps[:], lhsT=w2p[:, f, :], rhs=hts[fc][:],
                                     start=(fc == 0), stop=(fc == 31))
            ot = outp.tile([128, 512], F32, tag="out")
            nc.vector.tensor_add(ot[:], x2f[cc][:], ps[:])
            nc.sync.dma_start(out=out_d.ap()[128 * cc:128 * (cc + 1), :], in_=ot[:])

    outer.close()


def build(single_core=False, stop_after=None, repeats=1):
    global _PROGRAM
    if not single_core and repeats == 1 and _PROGRAM is not None:
        return _PROGRAM
    nc = bacc.Bacc("TRN2", target_bir_lowering=False, debug=False,
                   num_devices=1 if single_core else NCORES)
    io = {
        "xT": nc.dram_tensor("xT", [C, NT], F32, kind="ExternalInput"),
        "xTown": nc.dram_tensor("xTown", [C, TOK], F32, kind="ExternalInput"),
        "wqkv": nc.dram_tensor("wqkv", [C, 384], F32, kind="ExternalInput"),
        "wproj": nc.dram_tensor("wproj", [C, C], F32, kind="ExternalInput"),
        "w1t": nc.dram_tensor("w1t", [C, FF], F32, kind="ExternalInput"),
        "w2t": nc.dram_tensor("w2t", [FF, C], F32, kind="ExternalInput"),
        "b1": nc.dram_tensor("b1", [FF, 1], F32, kind="ExternalInput"),
        "masks": nc.dram_tensor("masks", [128, 4, 512], F32, kind="ExternalInput"),
        "out": nc.dram_tensor("out", [C, TOK], F32, kind="ExternalOutput"),
    }
    with tile.TileContext(nc) as tc:
        for _r in range(repeats):
            _emit(nc, tc, io, use_collective=not single_core,
                  stop_after=stop_after)
    nc.compile()
    if single_core or repeats != 1:
        return nc
    _PROGRAM = nc
    return nc


def kernel(x, Wq, Wk, Wv, Wproj, W1, b1, W2):
    global LAST_EXEC_NS
    x = np.asarray(x, np.float32)
    xT = np.ascontiguousarray(x.reshape(NT, C).T)
    wprojT = np.ascontiguousarray(np.asarray(Wproj, np.float32).T)
    w1t = np.ascontiguousarray(np.asarray(W1, np.float32).T)
    w2t = np.ascontiguousarray(np.asarray(W2, np.float32).T)
    b1v = np.ascontiguousarray(np.asarray(b1, np.float32).reshape(FF, 1))
    Wq = np.asarray(Wq, np.float32)
    Wk = np.asarray(Wk, np.float32)
    Wv = np.asarray(Wv, np.float32)

    s_i = np.arange(128)[:, None, None]
    kr_i = np.arange(4)[None, :, None]
    t_i = np.arange(512)[None, None, :]
    masks = (128 * kr_i + s_i <= t_i).astype(np.float32)

    in_maps = []
    for c in range(NCORES):
        h0, h1 = 2 * c, 2 * c + 1
        wqkv = np.ascontiguousarray(np.concatenate(
            [Wq[h0], Wq[h1], Wk[h0], Wk[h1], Wv[h0], Wv[h1]], axis=1))
        in_maps.append({
            "xT": xT,
            "xTown": np.ascontiguousarray(xT[:, TOK * c:TOK * (c + 1)]),
            "wqkv": wqkv,
            "wproj": wprojT, "w1t": w1t, "w2t": w2t, "b1": b1v,
            "masks": masks,
        })

    nc = build()
    res = bass_utils.run_bass_kernel_spmd(
        nc, in_maps, core_ids=list(range(NCORES)))

    full = np.empty((NT, C), np.float32)
    for c in range(NCORES):
        full[TOK * c:TOK * (c + 1), :] = res.results[c]["out"].T
    return full.reshape(B, T, C)

